# revision 39
# baseline (speedup 1.0000x reference)
"""Trainium2 Bass kernel for MultiHeadAttention with RoPE.

Problem: B=2, L=2048, d_model=1024, 16 heads, d_k=64, fp32 in/out.

Sharding (8 cores): batch x head-group.  Core c owns batch c//4 and the 4
heads 4*(c%4)..4*(c%4)+3 (a 256-wide slice of the projection dims).  Each
core reads only its batch's q/k/v (transposed + bf16 on host), its 256-row
slice of Wq/Wk/Wv (pre-transposed) and the matching 256 columns of Wo.
The host sums the 4 partial outputs per batch and adds bo.

Per-core pipeline (all matmuls bf16, fp32 PSUM accumulation):
  * inputs are staged ONCE on the sync HWDGE ring in order q -> k -> v
    (consts on the scalar HWDGE ring); both head-group m-tiles project
    from the same staged tiles, as 4 sequential (mt, j) PSUM streams per
    input, each evicted as soon as its contraction finishes.
  * RoPE: rotate-half via two strided partition-swap SWDGE copies + 3 DVE
    ops per (mt, j) half; 1/sqrt(dk) and the sign fold into host tables.
  * block (0,0) runs scores/exp ONLY (all 16 ex tiles kept) while vT is
    still streaming; v-proj (DVE evictions), the m-tile-0 vaug transposes
    and the deferred ctx accumulation then run under that exp stream.
  * scores per head via SAME-HEAD block-diagonal packing: chunk c is
    diag(kh[:, 128c:128c+64], kh[:, 128c+64:128c+128]) against a
    partition-duplicated q rhs -> PSUM [128 contiguous kt, qt].
  * exp on ScalarE (no max-subtract: scores ~ N(0,1)), bf16 out.  During
    attention the ScalarE queue holds NOTHING but exps (all evictions DVE,
    den extraction via DVE, rope shifts via SWDGE), so exp paces.
  * ctx per head: stationary vaug [128 kt, 65] whose 65th column is ones
    -> row 64 of the PSUM accumulator is the softmax denominator free.
  * normalize straight from PSUM: den row -> reciprocal -> gpsimd
    partition-broadcast -> DVE multiply; each block's ctx tail + normalize
    is deferred past the NEXT block's first exp (stagger) so the exp
    stream never waits.
  * out-proj [tok, 1024] = ctx @ WoT: j0 token tiles trickle through the
    j1 blocks at one matmul per two chunks (inside the PE slack), j1
    tiles pipeline on the idle scores ring at the end.

PSUM budget (16KB/partition): tag "mm" 2x4KB (scores / q,k proj / tail
out-proj), tag "cp" 1x4KB (ctx accumulators / v-proj), tag "po" 1x4KB
(vaug transposes / v-proj / out-proj fills).
"""

import numpy as np
import ml_dtypes

import concourse.bass as bass
import concourse.mybir as mybir
import concourse.tile as tile
from concourse import bacc
from concourse.bass_utils import run_bass_kernel_spmd

BF = mybir.dt.bfloat16
F32 = mybir.dt.float32
AF = mybir.ActivationFunctionType
ALU = mybir.AluOpType

NCORES = 8
B = 2
L = 2048
D = 1024          # d_model
H = 16            # heads
DK = 64           # head dim
HPC = 4           # heads per core
PD = HPC * DK     # projection dims per core = 256
TOK = L           # tokens per core (one batch)
P = 128
NMT = PD // P     # matmul M-tiles per projection = 2

ROPE_BASE = 10000.0


def build_nc(debug_dumps=False):
    """Build the single-core Bass program (SPMD: same program, per-core data)."""
    from contextlib import ExitStack

    nc = bacc.Bacc("TRN2", target_bir_lowering=False, debug=False)

    # ---- DRAM I/O ----
    qT = nc.dram_tensor("qT", [D, TOK], BF, kind="ExternalInput").ap()
    kT = nc.dram_tensor("kT", [D, TOK], BF, kind="ExternalInput").ap()
    vT = nc.dram_tensor("vT", [D, TOK], BF, kind="ExternalInput").ap()
    wqT = nc.dram_tensor("wqT", [D, PD], BF, kind="ExternalInput").ap()
    wkT = nc.dram_tensor("wkT", [D, PD], BF, kind="ExternalInput").ap()
    wvT = nc.dram_tensor("wvT", [D, PD], BF, kind="ExternalInput").ap()
    woT = nc.dram_tensor("woT", [PD, D], BF, kind="ExternalInput").ap()
    bq_d = nc.dram_tensor("bq", [PD, 1], F32, kind="ExternalInput").ap()
    bk_d = nc.dram_tensor("bk", [PD, 1], F32, kind="ExternalInput").ap()
    bv_d = nc.dram_tensor("bv", [PD, 1], F32, kind="ExternalInput").ap()
    cos_q = nc.dram_tensor("cos_q", [P, L], BF, kind="ExternalInput").ap()
    sin_q = nc.dram_tensor("sin_q", [P, L], BF, kind="ExternalInput").ap()
    cos_k = nc.dram_tensor("cos_k", [P, L], BF, kind="ExternalInput").ap()
    sin_k = nc.dram_tensor("sin_k", [P, L], BF, kind="ExternalInput").ap()
    outp = nc.dram_tensor("outp", [TOK, D], BF, kind="ExternalOutput").ap()

    with tile.TileContext(nc) as tc, ExitStack() as ctx:
        const = ctx.enter_context(tc.tile_pool(name="const", bufs=1))
        persist = ctx.enter_context(tc.tile_pool(name="persist", bufs=1))
        stage = ctx.enter_context(tc.tile_pool(name="stage", bufs=8))
        raws = ctx.enter_context(tc.tile_pool(name="raws", bufs=4))
        rots = ctx.enter_context(tc.tile_pool(name="rots", bufs=4))
        expp = ctx.enter_context(tc.tile_pool(name="expp", bufs=4))
        outs = ctx.enter_context(tc.tile_pool(name="outs", bufs=2))
        smalls = ctx.enter_context(tc.tile_pool(name="smalls", bufs=1))
        psum = ctx.enter_context(tc.tile_pool(name="psum", bufs=2, space="PSUM"))

        # ---- consts: the q/k path (weights, biases, rope tables) rides
        # FIRST on the sync HWDGE ring ahead of the q/k token stream (the
        # scalar ring is starved by the sync ring's round-robin share);
        # the late v/wo consts go on the scalar ring ----
        def load_w(name, w_d, eng):
            w_sb = const.tile([P, 8 * PD], BF, name=name)
            eng.dma_start(
                w_sb.rearrange("p (a m) -> p a m", a=8),
                w_d.rearrange("(a p) m -> p a m", p=P),
            )
            return w_sb

        def load_b(name, b_d, eng):
            b_sb = const.tile([P, NMT], F32, name=name)
            eng.dma_start(
                b_sb.rearrange("p (a m) -> p a m", a=NMT),
                b_d.rearrange("(a p) m -> p a m", p=P),
            )
            return b_sb

        def load_c(name, t_d, eng):
            t_sb = const.tile([P, L], BF, name=name)
            eng.dma_start(t_sb[:], t_d[:])
            return t_sb

        wq_sb = load_w("wq_sb", wqT, nc.sync)
        bq_sb = load_b("bq_sb", bq_d, nc.sync)
        cq_sb = load_c("cq_sb", cos_q, nc.sync)
        sq_sb = load_c("sq_sb", sin_q, nc.sync)
        wk_sb = load_w("wk_sb", wkT, nc.sync)
        bk_sb = load_b("bk_sb", bk_d, nc.sync)
        ck_sb = load_c("ck_sb", cos_k, nc.sync)
        sk_sb = load_c("sk_sb", sin_k, nc.sync)
        wv_sb = load_w("wv_sb", wvT, nc.scalar)
        bv_sb = load_b("bv_sb", bv_d, nc.scalar)
        wo_sb = [const.tile([P, D], BF, name=f"wo_{m}") for m in range(NMT)]
        for m in range(NMT):
            nc.scalar.dma_start(wo_sb[m][:], woT[m * P:(m + 1) * P, :])

        # ---- t0 engine-local setup ----
        ident = const.tile([P, P], BF)
        from concourse.masks import make_identity
        make_identity(nc, ident[:])
        ones_sb = const.tile([P, 1024], BF)
        nc.vector.memset(ones_sb[:], 1.0)

        # persistent per-head attention operands.  vaug packs the two heads
        # of one m-tile as [128 kt, 16 chunks, 2*65] so each PE transpose
        # needs a single strided eviction copy.
        qs2 = [persist.tile([P, L], BF, name=f"qs2_{h}") for h in range(HPC)]
        kh2 = [persist.tile([P, L], BF, name=f"kh2_{h}") for h in range(HPC)]
        vaug = [persist.tile([P, 16 * 130], BF, name=f"vaug_{m}")
                for m in range(NMT)]
        vh_sb = [persist.tile([P, L], BF, name=f"vh_{m}") for m in range(NMT)]
        ctx_sb = [persist.tile([P, L], BF, name=f"ctx_{m}") for m in range(NMT)]

        def vaug_h(h, c):
            """lhsT [128 kt, 65] for head h, kt chunk c (65th col = ones)."""
            o = c * 130 + (h % 2) * 65
            return vaug[h // 2][:, o:o + 65]

        for t in kh2:
            nc.gpsimd.memset(t[:], 0.0)
        for t in vaug:
            nc.vector.memset(
                t.rearrange("p (c g u) -> p c g u", g=2, u=65)[:, :, :, 64:65],
                1.0)

        # warm the ScalarE exp table set before the attention phase
        dummy = smalls.tile([1, 8], F32, name="dummy", tag="dummy")
        nc.vector.memset(dummy[:], 0.0)
        nc.scalar.activation(dummy[:], dummy[:], AF.Exp)

        # ---- input stream: all x tiles on the sync ring, q -> k -> v ----
        xs = {}
        for name, x_d in (("q", qT), ("k", kT), ("v", vT)):
            xs[name] = [stage.tile([P, L], BF, name=f"x_{name}", tag="stage")
                        for _ in range(8)]
            for kc in range(8):
                nc.sync.dma_start(xs[name][kc][:], x_d[kc * P:(kc + 1) * P, :])

        # ---- projections: kc-outer over concurrent (mt, j) PSUM streams,
        # so the matmuls track tile arrival ----
        def proj_mm(kind, w_sb, streams):
            """streams: list of (mt, j, tag, bufs); all advance kc-outer.
            Returns the PSUM stream dict (evict separately)."""
            w_r = w_sb.rearrange("p (a m) -> p a m", a=8)
            ps = {}
            for (mt, j, tag, bufs) in streams:
                ps[(mt, j)] = psum.tile([P, 1024], F32,
                                        name=f"p{kind}{mt}{j}",
                                        tag=tag, bufs=bufs)
            for kc in range(8):
                for (mt, j, tag, bufs) in streams:
                    for nb in range(2):
                        c0 = j * 1024 + nb * 512
                        nc.tensor.matmul(
                            ps[(mt, j)][:, nb * 512:(nb + 1) * 512],
                            lhsT=w_r[:, kc, mt * P:(mt + 1) * P],
                            rhs=xs[kind][kc][:, c0:c0 + 512],
                            start=(kc == 0), stop=(kc == 7),
                            skip_group_check=True,
                        )
            return ps

        QK_STREAMS = [(0, 0, "mm", None), (0, 1, "mm", None),
                      (1, 0, "cp", 1), (1, 1, "po", 1)]

        def evict_qk(kind, bias_sb, ps, mt, j):
            """ScalarE eviction (pre-attention only) -> fresh raw tile."""
            rawj = raws.tile([P, 1024], BF, name=f"raw{kind}{mt}{j}",
                             tag="raw")
            nc.scalar.activation(rawj[:], ps[(mt, j)][:], AF.Identity,
                                 bias=bias_sb[:, mt:mt + 1])
            return rawj

        def rope_chain(kind, cos_sb, sin_sb, pack, mt, j, rawj):
            """Rotate-half + scale + pack for one (mt, j) half."""
            jsl = slice(j * 1024, (j + 1) * 1024)
            rotj = rots.tile([P, 1024], BF, name=f"rot{kind}{mt}{j}",
                             tag="rot")
            for hb in range(2):
                r0 = hb * DK
                nc.gpsimd.dma_start(rotj[r0:r0 + 32, :],
                                    rawj[r0 + 32:r0 + 64, :])
                nc.gpsimd.dma_start(rotj[r0 + 32:r0 + 64, :],
                                    rawj[r0:r0 + 32, :])
            nc.vector.tensor_mul(rawj[:], rawj[:], cos_sb[:, jsl])
            nc.vector.tensor_mul(rotj[:], rotj[:], sin_sb[:, jsl])
            nc.vector.tensor_add(rotj[:], rotj[:], rawj[:])
            pack(mt, j, rotj)

        def pack_q(mt, j, rotj):
            jsl = slice(j * 1024, (j + 1) * 1024)
            for hl in range(2):
                h = 2 * mt + hl
                src = rotj[hl * DK:(hl + 1) * DK, :]
                nc.vector.tensor_copy(qs2[h][0:DK, jsl], src)
                nc.vector.tensor_copy(qs2[h][DK:P, jsl], src)

        def pack_k(mt, j, rotj):
            csl = slice(j * 8, (j + 1) * 8)
            for hl in range(2):
                h = 2 * mt + hl
                src = rotj[hl * DK:(hl + 1) * DK, :].rearrange(
                    "p (c g u) -> p c g u", g=2, u=DK)
                dst = kh2[h].rearrange("p (c g u) -> p c g u", g=2, u=DK)
                nc.vector.tensor_copy(dst[0:DK, csl, 0, :], src[:, :, 0, :])
                nc.vector.tensor_copy(dst[DK:P, csl, 1, :], src[:, :, 1, :])

        # q/k projection + rope.  DVE/gpsimd chain order is what gates the
        # first exp: q m-tile 0 first, then k m-tile 0 (kh2 for heads 0/1),
        # deferring q/k m-tile 1 (their evictions still run early to free
        # the PSUM rings for the k matmuls / first score tiles).
        ps_q = proj_mm("q", wq_sb, QK_STREAMS)
        q_raw = {}
        for (mt, j) in [(0, 0), (0, 1), (1, 0), (1, 1)]:
            q_raw[(mt, j)] = evict_qk("q", bq_sb, ps_q, mt, j)
            if mt == 0:
                rope_chain("q", cq_sb, sq_sb, pack_q, mt, j, q_raw[(mt, j)])
        ps_k = proj_mm("k", wk_sb, QK_STREAMS)
        k_raw = {}
        for (mt, j) in [(0, 0), (0, 1)]:
            k_raw[(mt, j)] = evict_qk("k", bk_sb, ps_k, mt, j)
            rope_chain("k", ck_sb, sk_sb, pack_k, mt, j, k_raw[(mt, j)])
        for (mt, j) in [(1, 0), (1, 1)]:
            rope_chain("q", cq_sb, sq_sb, pack_q, mt, j, q_raw[(mt, j)])
            k_raw[(mt, j)] = evict_qk("k", bk_sb, ps_k, mt, j)
            rope_chain("k", ck_sb, sk_sb, pack_k, mt, j, k_raw[(mt, j)])

        def v_sink(mt, j, ps):
            # DVE eviction: the ScalarE queue is already full of exps here
            nc.vector.scalar_tensor_tensor(
                vh_sb[mt][:, j * 1024:(j + 1) * 1024], ps[:],
                bv_sb[:, mt:mt + 1], ones_sb[:], ALU.add, ALU.mult)

        def vaug_steps(mt, tags):
            """PE-transpose vh [dims, kt] -> vaug [kt, dims]: one strided
            DVE copy per transpose; pt tiles rotate over `tags`."""
            dst = vaug[mt].rearrange("p (c g u) -> p c g u", g=2, u=65)
            def tr(c, mt=mt):
                tag, bufs = tags[c % len(tags)]
                pt = psum.tile([P, P], BF, name="pt", tag=tag, bufs=bufs)
                nc.tensor.transpose(
                    pt[:], vh_sb[mt][:, c * P:(c + 1) * P], ident[:])
                src = pt.rearrange("p (g u) -> p g u", g=2)
                nc.vector.tensor_copy(dst[:, c, :, 0:DK], src[:])
            return [lambda c=c: tr(c) for c in range(16)]

        # ---------- attention ----------
        fills = []

        def pump(n):
            for _ in range(n):
                if fills:
                    fills.pop(0)()

        def scores_chunk(h, j, c, extag="exp", exbufs=None):
            qs = qs2[h][:, j * 1024:(j + 1) * 1024]
            sc = psum.tile([P, 1024], F32, name="sc", tag="mm")
            for nb in range(2):
                nc.tensor.matmul(
                    sc[:, nb * 512:(nb + 1) * 512],
                    lhsT=kh2[h][:, c * P:(c + 1) * P],
                    rhs=qs[:, nb * 512:(nb + 1) * 512],
                    start=True, stop=True, skip_group_check=True,
                )
            ex = expp.tile([P, 1024], BF, name="ex", tag=extag, bufs=exbufs)
            nc.scalar.activation(ex[:], sc[:], AF.Exp)
            return ex

        def ctx_chunk(h, cp, c, ex):
            for nb in range(2):
                sl = slice(nb * 512, (nb + 1) * 512)
                nc.tensor.matmul(
                    cp[:, sl], lhsT=vaug_h(h, c), rhs=ex[:, sl],
                    start=(c == 0), stop=(c == 15), skip_group_check=True,
                )

        def normalize(h, j, cp, pe_bcast=False):
            """ctx_sb rows = cfull[0:64] * broadcast(1/cfull[64]).  The
            cfull staging copy releases cp right away so the next block's
            ctx accumulation never waits on this chain.  The broadcast is
            gpsimd mid-stream (hidden under exps) but a PE outer-product
            for the final block (shorter critical path)."""
            mt, hl = h // 2, h % 2
            if pe_bcast:
                # tail chain: read cp directly (its release time no longer
                # matters) and broadcast via a PE outer product
                den = smalls.tile([1, 1024], F32, name="den", tag="den")
                nc.vector.tensor_copy(den[:], cp[64:65, :])
                cfull = smalls.tile([DK, 1024], F32, name="cfull",
                                    tag="cfull")
                nc.vector.tensor_copy(cfull[:], cp[0:DK, :])
                rec = smalls.tile([1, 1024], F32, name="rec", tag="rec")
                nc.vector.reciprocal_approx_fast(rec[:], den[:])
                recb = smalls.tile([1, 1024], BF, name="recb", tag="recb")
                nc.vector.tensor_copy(recb[:], rec[:])
                bcs = psum.tile([DK, 1024], F32, name="bcsp", tag="mm",
                                bufs=None)
                for nb in range(2):
                    nc.tensor.matmul(
                        bcs[:, nb * 512:(nb + 1) * 512],
                        lhsT=ones_sb[0:1, 0:DK],
                        rhs=recb[:, nb * 512:(nb + 1) * 512],
                        start=True, stop=True, skip_group_check=True,
                    )
                nc.vector.tensor_mul(
                    ctx_sb[mt][hl * DK:(hl + 1) * DK,
                               j * 1024:(j + 1) * 1024],
                    cfull[:], bcs[:])
                return
            den = smalls.tile([1, 1024], F32, name="den", tag="den")
            nc.vector.tensor_copy(den[:], cp[64:65, :])
            cfull = smalls.tile([DK, 1024], F32, name="cfull", tag="cfull")
            nc.vector.tensor_copy(cfull[:], cp[0:DK, :])
            rec = smalls.tile([1, 1024], F32, name="rec", tag="rec")
            nc.vector.reciprocal_approx_fast(rec[:], den[:])
            bcs = smalls.tile([DK, 1024], F32, name="bcs", tag="bcs")
            nc.gpsimd.partition_broadcast(bcs[:], rec[:], channels=DK)
            nc.vector.tensor_mul(
                ctx_sb[mt][hl * DK:(hl + 1) * DK, j * 1024:(j + 1) * 1024],
                cfull[:], bcs[:])

        def attention(h, j, finish_prev, pump_every=2):
            """Standard block: scores/exp/ctx interleaved; returns a finish
            closure (last ctx chunk + normalize) that the caller runs after
            the NEXT block's first exp, so the exp stream never waits."""
            cp = psum.tile([65, 1024], F32, name="cp", tag="cp", bufs=1)
            st = {}
            st["ex"] = scores_chunk(h, j, 0)
            if finish_prev is not None:
                finish_prev()
            for c in range(1, 16):
                ex_n = scores_chunk(h, j, c)
                ctx_chunk(h, cp, c - 1, st["ex"])
                st["ex"] = ex_n
                if c % pump_every == 0:
                    pump(1)

            def finish(pe_bcast=False):
                ctx_chunk(h, cp, 15, st["ex"])
                normalize(h, j, cp, pe_bcast=pe_bcast)
            return finish

        def oproj_steps(tb, tag, bufs, evict_alt=False):
            """Out-projection for token tile tb as 4 single-MM steps +
            eviction, so fills fit inside the per-chunk PE slack."""
            t0 = tb * P
            st = {}

            def mm(i, tb=tb):
                mt, nb = i // 2, i % 2
                if i == 0:
                    st["po"] = psum.tile([P, D], F32, name="po", tag=tag,
                                         bufs=bufs)
                nc.tensor.matmul(
                    st["po"][:, nb * 512:(nb + 1) * 512],
                    lhsT=ctx_sb[mt][:, t0:t0 + P],
                    rhs=wo_sb[mt][:, nb * 512:(nb + 1) * 512],
                    start=(mt == 0), stop=(mt == NMT - 1),
                    skip_group_check=True,
                )
                if i == 3:
                    ob = outs.tile([P, D], BF, name="ob", tag="out")
                    if evict_alt and tb % 2 == 1:
                        nc.scalar.activation(ob[:], st["po"][:], AF.Identity)
                    else:
                        nc.vector.tensor_copy(ob[:], st["po"][:])
                    nc.sync.dma_start(outp[t0:t0 + P, :], ob[:])
            return [lambda i=i: mm(i) for i in range(4)]

        # ---- block (0,0): scores/exp only (its 16 ex tiles persist on the
        # dedicated "exp0" ring) while vT streams in.  v-proj runs in two
        # kc-outer passes on the cp/po rings under the exp stream, then the
        # m-tile-0 vaug transposes fill the gap before block (1,0).  The
        # deferred ctx(0,0) chunks trickle through blocks (2,0)/(3,0)/(0,1)
        # as fills on the "po" ring. ----
        exs0 = [scores_chunk(0, 0, c, extag="exp0", exbufs=16)
                for c in range(16)]
        for mt in range(NMT):
            ps_v = proj_mm("v", wv_sb, [(mt, 0, "cp", 1), (mt, 1, "po", 1)])
            v_sink(mt, 0, ps_v[(mt, 0)])
            v_sink(mt, 1, ps_v[(mt, 1)])
        for step in vaug_steps(0, [("cp", 1), ("po", 1)]):
            step()

        def burst_steps():
            st = {}
            def mk(c):
                if c == 0:
                    st["cp0"] = psum.tile([65, 1024], F32, name="cp0",
                                          tag="po", bufs=1)
                ctx_chunk(0, st["cp0"], c, exs0[c])
                if c == 15:
                    normalize(0, 0, st["cp0"])
            return [lambda c=c: mk(c) for c in range(16)]

        # fills: vaug m-tile 1 (on the "mm" ring, interleaving with the
        # score tiles of block (1,0)), then the deferred ctx(0,0) chunks,
        # then the j0-token out-projection tiles.
        fills.extend(vaug_steps(1, [("mm", None)]))
        fin = attention(1, 0, None, pump_every=1)
        fills.extend(burst_steps())
        for (h, j) in [(2, 0), (3, 0), (0, 1), (1, 1), (2, 1), (3, 1)]:
            if (h, j) == (0, 1):
                for tb in range(8):
                    fills.extend(oproj_steps(tb, "po", 1))
            fin = attention(h, j, fin)
        fin(pe_bcast=True)
        # leftover fills keep the PE warm while the final normalize runs
        pump(len(fills))
        # tail token tiles pipeline on the idle "mm" ring
        for tb in range(8, 16):
            for step in oproj_steps(tb, "mm", None, evict_alt=True):
                step()

    return nc


def _rope_tables():
    """Host-built RoPE tables, transposed to [d, t], 2 heads stacked.

    sin is sign-folded for the rotate-half convention; q tables carry the
    1/sqrt(dk) attention scale.
    """
    inv_freq = 1.0 / (ROPE_BASE ** (np.arange(0, DK, 2, dtype=np.float64) / DK))
    t = np.arange(L, dtype=np.float64)
    ang = np.outer(t, inv_freq)               # [L, 32]
    emb = np.concatenate([ang, ang], axis=1)  # [L, 64]
    cos = np.cos(emb).T.astype(np.float32)    # [64, L]
    sin = np.sin(emb).T.astype(np.float32)
    sin_folded = sin.copy()
    sin_folded[:32] *= -1.0
    scale = 1.0 / np.sqrt(DK)
    cos2 = np.concatenate([cos, cos], axis=0)                # [128, L]
    sin2 = np.concatenate([sin_folded, sin_folded], axis=0)  # [128, L]
    bf = ml_dtypes.bfloat16
    return (
        (cos2 * scale).astype(bf), (sin2 * scale).astype(bf),
        cos2.astype(bf), sin2.astype(bf),
    )


_NC_CACHE = {}


def _get_nc():
    if "nc" not in _NC_CACHE:
        nc = build_nc()
        nc.finalize()
        _NC_CACHE["nc"] = nc
    return _NC_CACHE["nc"]


def make_in_maps(q, k, v, Wq, bq, Wk, bk, Wv, bv, Wo, bo):
    bf = ml_dtypes.bfloat16
    cos_q, sin_q, cos_k, sin_k = _rope_tables()
    xT = {}
    for b in range(B):
        xT[("q", b)] = np.ascontiguousarray(np.asarray(q)[b].T).astype(bf)
        xT[("k", b)] = np.ascontiguousarray(np.asarray(k)[b].T).astype(bf)
        xT[("v", b)] = np.ascontiguousarray(np.asarray(v)[b].T).astype(bf)
    in_maps = []
    for c in range(NCORES):
        b, g = c // 4, c % 4
        hs = slice(g * PD, (g + 1) * PD)
        in_maps.append({
            "qT": xT[("q", b)], "kT": xT[("k", b)], "vT": xT[("v", b)],
            "wqT": np.ascontiguousarray(np.asarray(Wq)[hs, :].T).astype(bf),
            "wkT": np.ascontiguousarray(np.asarray(Wk)[hs, :].T).astype(bf),
            "wvT": np.ascontiguousarray(np.asarray(Wv)[hs, :].T).astype(bf),
            "woT": np.ascontiguousarray(np.asarray(Wo)[:, hs].T).astype(bf),
            "bq": np.asarray(bq[hs], np.float32).reshape(PD, 1),
            "bk": np.asarray(bk[hs], np.float32).reshape(PD, 1),
            "bv": np.asarray(bv[hs], np.float32).reshape(PD, 1),
            "cos_q": cos_q, "sin_q": sin_q, "cos_k": cos_k, "sin_k": sin_k,
        })
    return in_maps


def kernel(q, k, v, Wq, bq, Wk, bk, Wv, bv, Wo, bo):
    assert q.shape == (B, L, D) and k.shape == (B, L, D) and v.shape == (B, L, D)
    in_maps = make_in_maps(q, k, v, Wq, bq, Wk, bk, Wv, bv, Wo, bo)
    nc = _get_nc()
    res = run_bass_kernel_spmd(nc, in_maps, list(range(NCORES)))
    out = np.zeros((B, TOK, D), np.float64)
    for c, r in enumerate(res.results):
        out[c // 4] += r["outp"].astype(np.float64)
    out += np.asarray(bo, np.float64)[None, None, :]
    return out.astype(np.float32)


# revision 40
# speedup vs baseline: 1.0480x; 1.0480x over previous
"""Trainium2 Bass kernel for MultiHeadAttention with RoPE.

Problem: B=2, L=2048, d_model=1024, 16 heads, d_k=64, fp32 in/out.

Sharding (8 cores): batch x head-group.  Core c owns batch c//4 and the 4
heads 4*(c%4)..4*(c%4)+3 (a 256-wide slice of the projection dims).  Each
core reads only its batch's q/k/v (transposed + bf16 on host), its 256-row
slice of Wq/Wk/Wv (pre-transposed) and the matching 256 columns of Wo.
The host sums the 4 partial outputs per batch and adds bo.

Per-core pipeline (all matmuls bf16, fp32 PSUM accumulation):
  * the q/k-path consts ride FIRST on the sync HWDGE ring, then the token
    stream as [128, 1024] column-half tiles in order q-j0, k-j0, k-j1,
    q-j1, v-j0, v-j1, so the first exp fires right at the DMA floor
    (boot + consts + q + k-half); late consts (wv, wo, perm) go on the
    scalar ring.  Inputs are staged once; both head-group m-tiles project
    from the same staged tiles, kc-outer so matmuls track tile arrival.
  * RoPE rotate-half is a PE permutation matmul (lhsT = partition-swapped
    identity) -> PSUM, evicted fused with the sin multiply (DVE STT) --
    no DMA on the SDMA engines, which the input stream saturates.
  * block (0,0) runs scores/exp ONLY (its 16 ex tiles persist on the
    "exp0" ring) while v still streams; the v projections follow, the
    vaug transposes trickle through blocks (1,0)/(2,0) and the deferred
    ctx(0,0) chunks through (3,0)/(0,1), all inside the per-chunk PE
    slack under the exp stream.
  * scores per head via SAME-HEAD block-diagonal packing: chunk c is
    diag(kh[:, 128c:128c+64], kh[:, 128c+64:128c+128]) against a
    partition-duplicated q rhs -> PSUM [128 contiguous kt, qt].
  * exp on ScalarE (no max-subtract: scores ~ N(0,1)), bf16 out.  During
    attention the ScalarE queue holds NOTHING but exps.
  * ctx per head: stationary vaug [128 kt, 65] whose 65th column is ones
    -> row 64 of the PSUM accumulator is the softmax denominator free.
  * normalize: staging copy (releases the ctx accumulator immediately),
    reciprocal, gpsimd partition-broadcast, DVE multiply; each block's
    ctx tail + normalize is deferred past the NEXT block's first exp.
  * out-proj [tok, 1024] = ctx @ WoT: j0 token tiles trickle through the
    j1 blocks one matmul per two chunks; j1 tiles pipeline on the idle
    scores ring at the end behind a shortened PE-broadcast normalize.

PSUM (16KB/partition): tag "mm" 2x4KB (scores / q,k proj / rope perm /
tail out-proj), tag "cp" 1x4KB (ctx accumulators / q-j1,v proj), tag
"po" 1x4KB (vaug transposes / q-j1,v proj / ctx(0,0) / out-proj fills).
"""

import numpy as np
import ml_dtypes

import concourse.bass as bass
import concourse.mybir as mybir
import concourse.tile as tile
from concourse import bacc
from concourse.bass_utils import run_bass_kernel_spmd

BF = mybir.dt.bfloat16
F32 = mybir.dt.float32
AF = mybir.ActivationFunctionType
ALU = mybir.AluOpType

NCORES = 8
B = 2
L = 2048
D = 1024          # d_model
H = 16            # heads
DK = 64           # head dim
HPC = 4           # heads per core
PD = HPC * DK     # projection dims per core = 256
TOK = L           # tokens per core (one batch)
P = 128
NMT = PD // P     # matmul M-tiles per projection = 2

ROPE_BASE = 10000.0


def build_nc(debug_dumps=False):
    """Build the single-core Bass program (SPMD: same program, per-core data)."""
    from contextlib import ExitStack

    nc = bacc.Bacc("TRN2", target_bir_lowering=False, debug=False)

    # ---- DRAM I/O ----
    qT = nc.dram_tensor("qT", [D, TOK], BF, kind="ExternalInput").ap()
    kT = nc.dram_tensor("kT", [D, TOK], BF, kind="ExternalInput").ap()
    vT = nc.dram_tensor("vT", [D, TOK], BF, kind="ExternalInput").ap()
    wqT = nc.dram_tensor("wqT", [D, PD], BF, kind="ExternalInput").ap()
    wkT = nc.dram_tensor("wkT", [D, PD], BF, kind="ExternalInput").ap()
    wvT = nc.dram_tensor("wvT", [D, PD], BF, kind="ExternalInput").ap()
    woT = nc.dram_tensor("woT", [PD, D], BF, kind="ExternalInput").ap()
    bq_d = nc.dram_tensor("bq", [PD, 1], F32, kind="ExternalInput").ap()
    bk_d = nc.dram_tensor("bk", [PD, 1], F32, kind="ExternalInput").ap()
    bv_d = nc.dram_tensor("bv", [PD, 1], F32, kind="ExternalInput").ap()
    cos_q = nc.dram_tensor("cos_q", [P, L], BF, kind="ExternalInput").ap()
    sin_q = nc.dram_tensor("sin_q", [P, L], BF, kind="ExternalInput").ap()
    cos_k = nc.dram_tensor("cos_k", [P, L], BF, kind="ExternalInput").ap()
    sin_k = nc.dram_tensor("sin_k", [P, L], BF, kind="ExternalInput").ap()
    outp = nc.dram_tensor("outp", [TOK, D], BF, kind="ExternalOutput").ap()

    with tile.TileContext(nc) as tc, ExitStack() as ctx:
        const = ctx.enter_context(tc.tile_pool(name="const", bufs=1))
        persist = ctx.enter_context(tc.tile_pool(name="persist", bufs=1))
        stage = ctx.enter_context(tc.tile_pool(name="stage", bufs=16))
        raws = ctx.enter_context(tc.tile_pool(name="raws", bufs=4))
        rots = ctx.enter_context(tc.tile_pool(name="rots", bufs=4))
        expp = ctx.enter_context(tc.tile_pool(name="expp", bufs=4))
        outs = ctx.enter_context(tc.tile_pool(name="outs", bufs=2))
        smalls = ctx.enter_context(tc.tile_pool(name="smalls", bufs=1))
        psum = ctx.enter_context(tc.tile_pool(name="psum", bufs=2, space="PSUM"))

        # ---- q/k-path consts FIRST on the sync ring ----
        def load_w(name, w_d, eng):
            w_sb = const.tile([P, 8 * PD], BF, name=name)
            eng.dma_start(
                w_sb.rearrange("p (a m) -> p a m", a=8),
                w_d.rearrange("(a p) m -> p a m", p=P),
            )
            return w_sb

        def load_b(name, b_d, eng):
            b_sb = const.tile([P, NMT], F32, name=name)
            eng.dma_start(
                b_sb.rearrange("p (a m) -> p a m", a=NMT),
                b_d.rearrange("(a p) m -> p a m", p=P),
            )
            return b_sb

        def load_c(name, t_d, eng):
            t_sb = const.tile([P, L], BF, name=name)
            eng.dma_start(t_sb[:], t_d[:])
            return t_sb

        wq_sb = load_w("wq_sb", wqT, nc.sync)
        bq_sb = load_b("bq_sb", bq_d, nc.sync)
        cq_sb = load_c("cq_sb", cos_q, nc.sync)
        sq_sb = load_c("sq_sb", sin_q, nc.sync)
        wk_sb = load_w("wk_sb", wkT, nc.sync)
        bk_sb = load_b("bk_sb", bk_d, nc.sync)
        ck_sb = load_c("ck_sb", cos_k, nc.sync)
        sk_sb = load_c("sk_sb", sin_k, nc.sync)
        wv_sb = load_w("wv_sb", wvT, nc.scalar)
        bv_sb = load_b("bv_sb", bv_d, nc.scalar)
        wo_sb = [const.tile([P, D], BF, name=f"wo_{m}") for m in range(NMT)]
        for m in range(NMT):
            nc.scalar.dma_start(wo_sb[m][:], woT[m * P:(m + 1) * P, :])

        # ---- t0 engine-local setup ----
        ident = const.tile([P, P], BF)
        from concourse.masks import make_identity
        make_identity(nc, ident[:])
        ones_sb = const.tile([P, 1024], BF)
        nc.vector.memset(ones_sb[:], 1.0)
        # rotate-half permutation matrix: perm[p, :] = ident[swap32(p), :]
        perm_sb = const.tile([P, P], BF)
        for g in range(2):
            r0 = g * DK
            nc.scalar.dma_start(perm_sb[r0:r0 + 32, :],
                                ident[r0 + 32:r0 + 64, :])
            nc.scalar.dma_start(perm_sb[r0 + 32:r0 + 64, :],
                                ident[r0:r0 + 32, :])

        # persistent per-head attention operands.  vaug packs the two heads
        # of one m-tile as [128 kt, 16 chunks, 2*65] so each PE transpose
        # needs a single strided eviction copy.
        qs2 = [persist.tile([P, L], BF, name=f"qs2_{h}") for h in range(HPC)]
        kh2 = [persist.tile([P, L], BF, name=f"kh2_{h}") for h in range(HPC)]
        vaug = [persist.tile([P, 16 * 130], BF, name=f"vaug_{m}")
                for m in range(NMT)]
        vh_sb = [persist.tile([P, L], BF, name=f"vh_{m}") for m in range(NMT)]
        ctx_sb = [persist.tile([P, L], BF, name=f"ctx_{m}") for m in range(NMT)]

        def vaug_h(h, c):
            """lhsT [128 kt, 65] for head h, kt chunk c (65th col = ones)."""
            o = c * 130 + (h % 2) * 65
            return vaug[h // 2][:, o:o + 65]

        for t in kh2:
            nc.gpsimd.memset(t[:], 0.0)
        for t in vaug:
            nc.vector.memset(
                t.rearrange("p (c g u) -> p c g u", g=2, u=65)[:, :, :, 64:65],
                1.0)

        # warm the ScalarE exp table set before the attention phase
        dummy = smalls.tile([1, 8], F32, name="dummy", tag="dummy")
        nc.vector.memset(dummy[:], 0.0)
        nc.scalar.activation(dummy[:], dummy[:], AF.Exp)

        # ---- input stream: column-half tiles on the sync ring ----
        xs = {}
        for name, x_d, j in (("q", qT, 0), ("k", kT, 0), ("k", kT, 1),
                             ("q", qT, 1), ("v", vT, 0), ("v", vT, 1)):
            xs[(name, j)] = [stage.tile([P, 1024], BF, name=f"x{name}{j}",
                                        tag="stage") for _ in range(8)]
            for kc in range(8):
                nc.sync.dma_start(
                    xs[(name, j)][kc][:],
                    x_d[kc * P:(kc + 1) * P, j * 1024:(j + 1) * 1024])

        # ---- projections: one column-half, both m-tiles, kc-outer ----
        def proj_mm(kind, j, w_sb, tags):
            w_r = w_sb.rearrange("p (a m) -> p a m", a=8)
            ps = {}
            for mt in range(NMT):
                tag, bufs = tags[mt]
                ps[mt] = psum.tile([P, 1024], F32, name=f"p{kind}{mt}{j}",
                                   tag=tag, bufs=bufs)
            for kc in range(8):
                for mt in range(NMT):
                    for nb in range(2):
                        nc.tensor.matmul(
                            ps[mt][:, nb * 512:(nb + 1) * 512],
                            lhsT=w_r[:, kc, mt * P:(mt + 1) * P],
                            rhs=xs[(kind, j)][kc][:, nb * 512:(nb + 1) * 512],
                            start=(kc == 0), stop=(kc == 7),
                            skip_group_check=True,
                        )
            return ps

        MM2 = [("mm", None), ("mm", None)]
        CPO = [("cp", 1), ("po", 1)]

        def evict_act(kind, bias_sb, ps, mt):
            """ScalarE eviction (safe only before the exp stream)."""
            rawj = raws.tile([P, 1024], BF, name=f"raw{kind}{mt}", tag="raw")
            nc.scalar.activation(rawj[:], ps[mt][:], AF.Identity,
                                 bias=bias_sb[:, mt:mt + 1])
            return rawj

        def evict_stt(kind, bias_sb, ps, mt):
            """DVE eviction (used once the ScalarE queue carries exps)."""
            rawj = raws.tile([P, 1024], BF, name=f"raw{kind}{mt}", tag="raw")
            nc.vector.scalar_tensor_tensor(
                rawj[:], ps[mt][:], bias_sb[:, mt:mt + 1], ones_sb[:],
                ALU.add, ALU.mult)
            return rawj

        def rope_chain(kind, cos_sb, sin_sb, pack, mt, j, rawj):
            """Rotate-half via PE permutation + fused sin multiply."""
            jsl = slice(j * 1024, (j + 1) * 1024)
            rot_ps = psum.tile([P, 1024], F32, name=f"rp{kind}{mt}{j}",
                               tag="mm")
            for nb in range(2):
                nc.tensor.matmul(
                    rot_ps[:, nb * 512:(nb + 1) * 512],
                    lhsT=perm_sb[:],
                    rhs=rawj[:, nb * 512:(nb + 1) * 512],
                    start=True, stop=True, skip_group_check=True,
                )
            rotj = rots.tile([P, 1024], BF, name=f"rot{kind}{mt}", tag="rot")
            nc.vector.scalar_tensor_tensor(
                rotj[:], rot_ps[:], 1.0, sin_sb[:, jsl], ALU.mult, ALU.mult)
            nc.vector.tensor_mul(rawj[:], rawj[:], cos_sb[:, jsl])
            nc.vector.tensor_add(rotj[:], rotj[:], rawj[:])
            pack(mt, j, rotj)

        def pack_q(mt, j, rotj):
            jsl = slice(j * 1024, (j + 1) * 1024)
            for hl in range(2):
                h = 2 * mt + hl
                src = rotj[hl * DK:(hl + 1) * DK, :]
                nc.vector.tensor_copy(qs2[h][0:DK, jsl], src)
                nc.vector.tensor_copy(qs2[h][DK:P, jsl], src)

        def pack_k(mt, j, rotj):
            csl = slice(j * 8, (j + 1) * 8)
            for hl in range(2):
                h = 2 * mt + hl
                src = rotj[hl * DK:(hl + 1) * DK, :].rearrange(
                    "p (c g u) -> p c g u", g=2, u=DK)
                dst = kh2[h].rearrange("p (c g u) -> p c g u", g=2, u=DK)
                nc.vector.tensor_copy(dst[0:DK, csl, 0, :], src[:, :, 0, :])
                nc.vector.tensor_copy(dst[DK:P, csl, 1, :], src[:, :, 1, :])

        def qk_phase(kind, j, w_sb, b_sb, c_sb, s_sb, pack, tags, ev):
            ps = proj_mm(kind, j, w_sb, tags)
            for mt in range(NMT):
                rawj = ev(kind, b_sb, ps, mt)
                rope_chain(kind, c_sb, s_sb, pack, mt, j, rawj)

        qk_phase("q", 0, wq_sb, bq_sb, cq_sb, sq_sb, pack_q, MM2, evict_act)
        qk_phase("k", 0, wk_sb, bk_sb, ck_sb, sk_sb, pack_k, MM2, evict_act)
        qk_phase("k", 1, wk_sb, bk_sb, ck_sb, sk_sb, pack_k, MM2, evict_act)

        # ---------- attention primitives ----------
        fills = []

        def pump(n):
            for _ in range(n):
                if fills:
                    fills.pop(0)()

        def scores_chunk(h, j, c, extag="exp", exbufs=None):
            qs = qs2[h][:, j * 1024:(j + 1) * 1024]
            sc = psum.tile([P, 1024], F32, name="sc", tag="mm")
            for nb in range(2):
                nc.tensor.matmul(
                    sc[:, nb * 512:(nb + 1) * 512],
                    lhsT=kh2[h][:, c * P:(c + 1) * P],
                    rhs=qs[:, nb * 512:(nb + 1) * 512],
                    start=True, stop=True, skip_group_check=True,
                )
            ex = expp.tile([P, 1024], BF, name="ex", tag=extag, bufs=exbufs)
            nc.scalar.activation(ex[:], sc[:], AF.Exp)
            return ex

        def ctx_chunk(h, cp, c, ex):
            for nb in range(2):
                sl = slice(nb * 512, (nb + 1) * 512)
                nc.tensor.matmul(
                    cp[:, sl], lhsT=vaug_h(h, c), rhs=ex[:, sl],
                    start=(c == 0), stop=(c == 15), skip_group_check=True,
                )

        def normalize(h, j, cp, pe_bcast=False):
            mt, hl = h // 2, h % 2
            den = smalls.tile([1, 1024], F32, name="den", tag="den")
            nc.vector.tensor_copy(den[:], cp[64:65, :])
            cfull = smalls.tile([DK, 1024], F32, name="cfull", tag="cfull")
            nc.vector.tensor_copy(cfull[:], cp[0:DK, :])
            rec = smalls.tile([1, 1024], F32, name="rec", tag="rec")
            nc.vector.reciprocal_approx_fast(rec[:], den[:])
            if pe_bcast:
                recb = smalls.tile([1, 1024], BF, name="recb", tag="recb")
                nc.vector.tensor_copy(recb[:], rec[:])
                bcs = psum.tile([DK, 1024], F32, name="bcsp", tag="mm")
                for nb in range(2):
                    nc.tensor.matmul(
                        bcs[:, nb * 512:(nb + 1) * 512],
                        lhsT=ones_sb[0:1, 0:DK],
                        rhs=recb[:, nb * 512:(nb + 1) * 512],
                        start=True, stop=True, skip_group_check=True,
                    )
            else:
                bcs = smalls.tile([DK, 1024], F32, name="bcs", tag="bcs")
                nc.gpsimd.partition_broadcast(bcs[:], rec[:], channels=DK)
            nc.vector.tensor_mul(
                ctx_sb[mt][hl * DK:(hl + 1) * DK, j * 1024:(j + 1) * 1024],
                cfull[:], bcs[:])

        def attention(h, j, finish_prev, pump_every=2):
            """Standard block: scores/exp/ctx interleaved.  Fills pump
            between the scores and the ctx of each chunk so a fill feeding
            chunk c's ctx lands right before it.  Returns a finish closure
            (one fill + last ctx chunk + normalize) that the caller runs
            after the NEXT block's first exp."""
            cp = psum.tile([65, 1024], F32, name="cp", tag="cp", bufs=1)
            st = {}
            st["ex"] = scores_chunk(h, j, 0)
            if finish_prev is not None:
                finish_prev()
            for c in range(1, 16):
                ex_n = scores_chunk(h, j, c)
                if c % pump_every == 0:
                    pump(1)
                ctx_chunk(h, cp, c - 1, st["ex"])
                st["ex"] = ex_n

            def finish(pe_bcast=False):
                pump(1)
                ctx_chunk(h, cp, 15, st["ex"])
                normalize(h, j, cp, pe_bcast=pe_bcast)
            return finish

        def oproj_steps(tb, tag, bufs, evict_alt=False):
            t0 = tb * P
            st = {}

            def mm(i, tb=tb):
                mt, nb = i // 2, i % 2
                if i == 0:
                    st["po"] = psum.tile([P, D], F32, name="po", tag=tag,
                                         bufs=bufs)
                nc.tensor.matmul(
                    st["po"][:, nb * 512:(nb + 1) * 512],
                    lhsT=ctx_sb[mt][:, t0:t0 + P],
                    rhs=wo_sb[mt][:, nb * 512:(nb + 1) * 512],
                    start=(mt == 0), stop=(mt == NMT - 1),
                    skip_group_check=True,
                )
                if i == 3:
                    ob = outs.tile([P, D], BF, name="ob", tag="out")
                    if evict_alt and tb % 2 == 1:
                        nc.scalar.activation(ob[:], st["po"][:], AF.Identity)
                    else:
                        nc.vector.tensor_copy(ob[:], st["po"][:])
                    nc.sync.dma_start(outp[t0:t0 + P, :], ob[:])
            return [lambda i=i: mm(i) for i in range(4)]

        # ---- block (0,0): scores/exp only; its ex tiles persist ----
        exs0 = [scores_chunk(0, 0, c, extag="exp0", exbufs=16)
                for c in range(16)]

        # q-j1 (DVE evictions -- the exp stream owns ScalarE now), then v
        qk_phase("q", 1, wq_sb, bq_sb, cq_sb, sq_sb, pack_q, CPO, evict_stt)

        def v_sink(mt, j, ps):
            nc.vector.scalar_tensor_tensor(
                vh_sb[mt][:, j * 1024:(j + 1) * 1024], ps[mt][:],
                bv_sb[:, mt:mt + 1], ones_sb[:], ALU.add, ALU.mult)

        for j in range(2):
            ps_v = proj_mm("v", j, wv_sb, CPO)
            for mt in range(NMT):
                v_sink(mt, j, ps_v)

        def vaug_steps(mt):
            """PE-transpose vh [dims, kt] -> vaug [kt, dims]: one strided
            DVE copy per transpose; pt tiles on the "po" ring."""
            dst = vaug[mt].rearrange("p (c g u) -> p c g u", g=2, u=65)
            def tr(c, mt=mt):
                pt = psum.tile([P, P], BF, name="pt", tag="po", bufs=1)
                nc.tensor.transpose(
                    pt[:], vh_sb[mt][:, c * P:(c + 1) * P], ident[:])
                src = pt.rearrange("p (g u) -> p g u", g=2)
                nc.vector.tensor_copy(dst[:, c, :, 0:DK], src[:])
            return [lambda c=c: tr(c) for c in range(16)]

        def burst_steps():
            """Deferred ctx(0,0) accumulation on the "po" ring."""
            st = {}
            def mk(c):
                if c == 0:
                    st["cp0"] = psum.tile([65, 1024], F32, name="cp0",
                                          tag="po", bufs=1)
                ctx_chunk(0, st["cp0"], c, exs0[c])
                if c == 15:
                    normalize(0, 0, st["cp0"])
            return [lambda c=c: mk(c) for c in range(16)]

        # ---- remaining blocks with fills ----
        fills.extend(vaug_steps(0))
        fin = attention(1, 0, None, pump_every=1)
        fills.extend(vaug_steps(1))
        fin = attention(2, 0, fin, pump_every=1)
        fills.extend(burst_steps())
        fin = attention(3, 0, fin)
        for (h, j) in [(0, 1), (1, 1), (2, 1), (3, 1)]:
            if (h, j) == (0, 1):
                for tb in range(8):
                    fills.extend(oproj_steps(tb, "po", 1))
            fin = attention(h, j, fin)
        fin(pe_bcast=True)
        # leftover fills keep the PE warm while the final normalize runs
        pump(len(fills))
        # tail token tiles pipeline on the idle "mm" ring
        for tb in range(8, 16):
            for step in oproj_steps(tb, "mm", None, evict_alt=True):
                step()

    return nc


def _rope_tables():
    """Host-built RoPE tables, transposed to [d, t], 2 heads stacked.

    sin is sign-folded for the rotate-half convention; q tables carry the
    1/sqrt(dk) attention scale.
    """
    inv_freq = 1.0 / (ROPE_BASE ** (np.arange(0, DK, 2, dtype=np.float64) / DK))
    t = np.arange(L, dtype=np.float64)
    ang = np.outer(t, inv_freq)               # [L, 32]
    emb = np.concatenate([ang, ang], axis=1)  # [L, 64]
    cos = np.cos(emb).T.astype(np.float32)    # [64, L]
    sin = np.sin(emb).T.astype(np.float32)
    sin_folded = sin.copy()
    sin_folded[:32] *= -1.0
    scale = 1.0 / np.sqrt(DK)
    cos2 = np.concatenate([cos, cos], axis=0)                # [128, L]
    sin2 = np.concatenate([sin_folded, sin_folded], axis=0)  # [128, L]
    bf = ml_dtypes.bfloat16
    return (
        (cos2 * scale).astype(bf), (sin2 * scale).astype(bf),
        cos2.astype(bf), sin2.astype(bf),
    )


_NC_CACHE = {}


def _get_nc():
    if "nc" not in _NC_CACHE:
        nc = build_nc()
        nc.finalize()
        _NC_CACHE["nc"] = nc
    return _NC_CACHE["nc"]


def make_in_maps(q, k, v, Wq, bq, Wk, bk, Wv, bv, Wo, bo):
    bf = ml_dtypes.bfloat16
    cos_q, sin_q, cos_k, sin_k = _rope_tables()
    xT = {}
    for b in range(B):
        xT[("q", b)] = np.ascontiguousarray(np.asarray(q)[b].T).astype(bf)
        xT[("k", b)] = np.ascontiguousarray(np.asarray(k)[b].T).astype(bf)
        xT[("v", b)] = np.ascontiguousarray(np.asarray(v)[b].T).astype(bf)
    in_maps = []
    for c in range(NCORES):
        b, g = c // 4, c % 4
        hs = slice(g * PD, (g + 1) * PD)
        in_maps.append({
            "qT": xT[("q", b)], "kT": xT[("k", b)], "vT": xT[("v", b)],
            "wqT": np.ascontiguousarray(np.asarray(Wq)[hs, :].T).astype(bf),
            "wkT": np.ascontiguousarray(np.asarray(Wk)[hs, :].T).astype(bf),
            "wvT": np.ascontiguousarray(np.asarray(Wv)[hs, :].T).astype(bf),
            "woT": np.ascontiguousarray(np.asarray(Wo)[:, hs].T).astype(bf),
            "bq": np.asarray(bq[hs], np.float32).reshape(PD, 1),
            "bk": np.asarray(bk[hs], np.float32).reshape(PD, 1),
            "bv": np.asarray(bv[hs], np.float32).reshape(PD, 1),
            "cos_q": cos_q, "sin_q": sin_q, "cos_k": cos_k, "sin_k": sin_k,
        })
    return in_maps


def kernel(q, k, v, Wq, bq, Wk, bk, Wv, bv, Wo, bo):
    assert q.shape == (B, L, D) and k.shape == (B, L, D) and v.shape == (B, L, D)
    in_maps = make_in_maps(q, k, v, Wq, bq, Wk, bk, Wv, bv, Wo, bo)
    nc = _get_nc()
    res = run_bass_kernel_spmd(nc, in_maps, list(range(NCORES)))
    out = np.zeros((B, TOK, D), np.float64)
    for c, r in enumerate(res.results):
        out[c // 4] += r["outp"].astype(np.float64)
    out += np.asarray(bo, np.float64)[None, None, :]
    return out.astype(np.float32)
